# revision 28
# baseline (speedup 1.0000x reference)
"""GCNII conv (gnn_message_passing) Trainium2 Bass kernel.

Strategy (8-way node sharding, dual-view HBM gather):
  - Host: per core, relabel the 40000 node rows into two overlapping views of
    exactly 32768 rows each (int16-addressable): the 25536 most-referenced
    rows appear in BOTH views; the 7232 coldest rows are A-only and the next
    7232 coldest B-only.  Each node's 16 neighbor refs then split exactly
    8 + 8 between the views (cold-row refs are ~0.3/node, so the 8-cap never
    binds) -- no padding slots, no zero row.
  - Device: dma_gather straight from the HBM view tables (bf16, 256B rows)
    in transpose mode, 896-idx chunks (hard ucode ring limit) round-robined
    over 2 SWDGE queues so descriptor generation runs on two Q7 core pairs.
    448-node tiles make each grid exactly 4 chunks; the last tile is 80
    nodes so the serial tail after the final gather is short.  The GCNII
    combine is folded into the slot sums: psum += M1s @ G_s for each of the
    16 slots plus M1s @ x_self and M2 @ x_0, all bf16 matmuls accumulating
    in fp32 PSUM, then bias + ReLU on the activation engine writing bf16
    output tiles (stored per tile).
"""

import numpy as np
import ml_dtypes

import concourse.bacc as bacc
import concourse.mybir as mybir
from concourse.tile import TileContext
from concourse.bass_utils import run_bass_kernel_spmd

BF16 = ml_dtypes.bfloat16
F32 = np.float32

ALPHA = 0.1
BETA = float(np.log(0.5 / 4 + 1.0))
DEG_K = 16           # neighbors per node (w/o self loop)
C = 128              # channels
P = 128              # partitions

# full-problem constants
N_FULL = 40000
N_CORES = 8
VIEW = 32768         # rows per view (int16 limit)
SLOTS = 8            # gather slots per node per view
import os as _os
SCRATCH = int(_os.environ.get("GCNII_SCRATCH", "16384"))   # ring bytes/partition
# Hard ucode limit: num_idxs/16 + 2 <= 64 ring descs per SDMA engine, so at
# most 992 idxs per dma_gather; 896 is the largest multiple of 128 below it.
GCH = int(_os.environ.get("GCNII_GCH", "896"))             # idxs per dma_gather
# 2 queues verified bit-exact on HW; 4 corrupts (Q7 cores 6-7 lack the
# extended addressing for queue 3's descriptor path), 3 wedges the device.
NQ = int(_os.environ.get("GCNII_NQ", "2"))                 # SWDGE queues (Q7 pairs)


def _tiles_for(nsh_pad):
    # 448-node tiles make each grid exactly 4 x 896-idx gather chunks (zero
    # ring-capacity rounding waste); the small last tile keeps the serial
    # tail after the final gather (matmuls + act + store) short.
    body, tail = divmod(nsh_pad, 448)
    tiles = [448] * body + ([tail] if tail else [])
    assert all(t % 16 == 0 for t in tiles) and sum(tiles) == nsh_pad
    return tiles


# --------------------------------------------------------------------------
# host-side preparation
# --------------------------------------------------------------------------

def _prep_core(x_bf16, idx_shard, n_rows, nsh_pad, tiles):
    """Build per-core dual-view table + per-tile slot grids.

    x_bf16:    [n_rows, C] bf16 node features (node-major)
    idx_shard: [nsh, K] global neighbor row ids for this core's nodes
    returns (table [2*VIEW, C] bf16, idx [P, nsh_pad] int16) where the idx
    columns for tile t hold grid A then grid B, each [16, SLOTS*nt/16]
    wrapped 16-partition slot-major and replicated x8 across partitions.
    """
    nsh, K = idx_shard.shape
    assert K == 2 * SLOTS
    n_mid = 2 * VIEW - n_rows          # rows present in both views
    n_cold = n_rows - n_mid            # split between A-only and B-only
    assert n_cold % 2 == 0
    half = n_cold // 2

    counts = np.bincount(idx_shard.reshape(-1), minlength=n_rows)
    order = np.argsort(-counts, kind="stable")   # hottest first
    mid_rows = order[:n_mid]
    a_rows = order[n_mid:n_mid + half]           # A-only
    b_rows = order[n_mid + half:]                # B-only

    lid_a = np.full(n_rows, -1, dtype=np.int64)
    lid_b = np.full(n_rows, -1, dtype=np.int64)
    lid_a[mid_rows] = np.arange(n_mid)
    lid_b[mid_rows] = np.arange(n_mid)
    lid_a[a_rows] = n_mid + np.arange(half)
    lid_b[b_rows] = n_mid + np.arange(half)

    table = np.zeros((2 * VIEW, C), dtype=BF16)
    table[lid_a[mid_rows]] = x_bf16[mid_rows]
    table[lid_a[a_rows]] = x_bf16[a_rows]
    table[VIEW + lid_b[mid_rows]] = x_bf16[mid_rows]
    table[VIEW + lid_b[b_rows]] = x_bf16[b_rows]

    # per-node 8/8 split: A-only refs must go to grid A, B-only to grid B,
    # middle refs fill the rest.
    la = lid_a[idx_shard]               # [nsh, K]; -1 where B-only
    lb = lid_b[idx_shard]
    a_only = lb < 0
    b_only = la < 0
    n_a = a_only.sum(axis=1)
    n_b = b_only.sum(axis=1)
    assert n_a.max() <= SLOTS and n_b.max() <= SLOTS, (n_a.max(), n_b.max())

    # order refs: A-only first, middle, B-only last; first 8 -> grid A.
    rank = np.where(a_only, 0, np.where(b_only, 2, 1))
    ord2 = np.argsort(rank, axis=1, kind="stable")
    la_s = np.take_along_axis(la, ord2, axis=1)
    lb_s = np.take_along_axis(lb, ord2, axis=1)
    slots_a = la_s[:, :SLOTS]
    slots_b = lb_s[:, SLOTS:]
    assert (slots_a >= 0).all() and (slots_b >= 0).all()

    if nsh_pad > nsh:
        pad = np.zeros((nsh_pad - nsh, SLOTS), dtype=np.int64)
        slots_a = np.concatenate([slots_a, pad], axis=0)
        slots_b = np.concatenate([slots_b, pad], axis=0)

    # per-tile idx block: [16, SLOTS*nt/16] per grid, A then B -> nt columns
    idx16 = np.zeros((16, nsh_pad), dtype=np.int16)
    off = 0
    for nt in tiles:
        sl = slice(off, off + nt)
        flat_a = slots_a[sl].T.reshape(-1)      # [SLOTS*nt] slot-major
        flat_b = slots_b[sl].T.reshape(-1)
        cpt = SLOTS * nt // 16
        idx16[:, off:off + cpt] = flat_a.reshape(-1, 16).T
        idx16[:, off + cpt:off + nt] = flat_b.reshape(-1, 16).T
        off += nt
    assert off == nsh_pad
    # ucode queue q reads indices from partitions 32q..32q+31 (Q7 cores
    # 2q, 2q+1); replicate for as many queues as the kernel uses.
    return table, np.tile(idx16, (2 * NQ, 1))


# --------------------------------------------------------------------------
# device program
# --------------------------------------------------------------------------

def _build_program(nsh_pad, tiles, repeat=1):
    dt = mybir.dt
    nc = bacc.Bacc("TRN2", target_bir_lowering=False, num_swdge_queues=NQ,
                   dynamic_dma_scratch_size=SCRATCH)

    tbl_d = nc.dram_tensor("tbl", [2 * VIEW, C], dt.bfloat16, kind="ExternalInput")
    idx_d = nc.dram_tensor("idx", [32 * NQ, nsh_pad], dt.int16, kind="ExternalInput")
    xx_d = nc.dram_tensor("xx", [P, 2 * nsh_pad], dt.bfloat16, kind="ExternalInput")
    m1t_d = nc.dram_tensor("m1t", [P, C], dt.bfloat16, kind="ExternalInput")
    m2t_d = nc.dram_tensor("m2t", [P, C], dt.bfloat16, kind="ExternalInput")
    bias_d = nc.dram_tensor("biasv", [P, 1], dt.float32, kind="ExternalInput")
    out_d = nc.dram_tensor("out", [P, nsh_pad], dt.bfloat16, kind="ExternalOutput")

    with TileContext(nc) as tc:
        with (
            tc.tile_pool(name="consts", bufs=1) as cpool,
            tc.tile_pool(name="ipool", bufs=4) as ipool,
            tc.tile_pool(name="xpool", bufs=4) as xpool,
            tc.tile_pool(name="gpool", bufs=3) as gpool,
            tc.tile_pool(name="opool", bufs=3) as opool,
            tc.tile_pool(name="psum", bufs=3, space="PSUM") as ppool,
        ):
          for _rep in range(repeat):
            m1t = cpool.tile([P, C], dt.bfloat16)
            m2t = cpool.tile([P, C], dt.bfloat16)
            biasv = cpool.tile([P, 1], dt.float32)

            # Dummy 128-idx gather per queue, issued first: absorbs the
            # one-time Q7 IRAM ucode load (~6us on HW, unmodeled by the cost
            # model) during the lead-in instead of delaying the first real
            # gather.  Indices come from a memset tile so no DMA dependency.
            ix0 = cpool.tile([P, 8], dt.int16)
            nc.vector.memset(ix0[:], 0)
            gwarm = cpool.tile([P, 1, NQ * 128], dt.bfloat16)
            for q in range(NQ):
                nc.gpsimd.dma_gather(
                    out_ap=gwarm[:, :, q * 128:(q + 1) * 128],
                    in_ap=tbl_d[0:VIEW, :],
                    idxs_ap=ix0[:, 0:8],
                    num_idxs=128,
                    num_idxs_reg=128,
                    elem_size=C,
                    transpose=True,
                    queue_num=q,
                )

            gq = 0       # round-robin SWDGE queue assignment
            off = 0
            for t, nt in enumerate(tiles):
                n0 = off
                n_gi = SLOTS * nt
                cpt = n_gi // 16

                ix = ipool.tile([P, nt], dt.int16)
                nc.scalar.dma_start(out=ix[0:32 * NQ, :], in_=idx_d[:, n0:n0 + nt])
                # partitions above 32*NQ are never read by the ucode (queue q
                # reads 32q..32q+31) but the interpreter checks full-tile
                # initialization; zero them on the otherwise-idle DVE.
                nc.vector.memset(ix[32 * NQ:P, :], 0)
                xxt = xpool.tile([P, 2 * nt], dt.bfloat16)
                nc.sync.dma_start(out=xxt[:], in_=xx_d[:, 2 * n0:2 * n0 + 2 * nt])

                g = gpool.tile([P, 1, 2 * n_gi], dt.bfloat16)
                for grid in (0, 1):
                    base = grid * n_gi
                    ibase = grid * cpt
                    view = tbl_d[0:VIEW, :] if grid == 0 else tbl_d[VIEW:2 * VIEW, :]
                    c0 = 0
                    while c0 < n_gi:
                        cn = min(GCH, n_gi - c0)
                        nc.gpsimd.dma_gather(
                            out_ap=g[:, :, base + c0:base + c0 + cn],
                            in_ap=view,
                            idxs_ap=ix[:, ibase + c0 // 16:ibase + (c0 + cn) // 16],
                            num_idxs=cn,
                            num_idxs_reg=cn,
                            elem_size=C,
                            transpose=True,
                            queue_num=gq % NQ,
                        )
                        gq += 1
                        c0 += cn
                if t == 0:
                    # weight/bias loads issued after the first gathers so
                    # they don't delay the first gather's DMA slot.
                    nc.scalar.dma_start(out=m1t[:], in_=m1t_d[:])
                    nc.scalar.dma_start(out=m2t[:], in_=m2t_d[:])
                    nc.scalar.dma_start(out=biasv[:], in_=bias_d[:])

                # x-matmuls first: they don't depend on the gathers, so only
                # the grid-B matmuls trail the tile's final gather.
                psum = ppool.tile([P, nt], dt.float32)
                nc.tensor.matmul(psum[:], lhsT=m1t[:],
                                 rhs=xxt[:, 0:nt],
                                 start=True, stop=False)
                nc.tensor.matmul(psum[:], lhsT=m2t[:],
                                 rhs=xxt[:, nt:2 * nt],
                                 start=False, stop=False)
                for s in range(2 * SLOTS):
                    nc.tensor.matmul(
                        psum[:], lhsT=m1t[:],
                        rhs=g[:, 0, s * nt:(s + 1) * nt],
                        start=False, stop=(s == 2 * SLOTS - 1))

                ot = opool.tile([P, nt], dt.bfloat16)
                nc.scalar.activation(
                    ot[:], psum[:],
                    mybir.ActivationFunctionType.Relu,
                    bias=biasv[:, 0:1], scale=1.0)
                nc.sync.dma_start(out=out_d[:, n0:n0 + nt], in_=ot[:])
                off += nt
    nc.compile()
    return nc


# --------------------------------------------------------------------------
# full host prep (shared by kernel() and tests)
# --------------------------------------------------------------------------

def _prepare(x, x_0, edge_index, W1, W2, bias, n_cores):
    x = np.asarray(x, dtype=F32)          # [1, C, N, 1]
    x_0 = np.asarray(x_0, dtype=F32)      # [1, N, C]
    ei = np.asarray(edge_index)           # [2, 1, N, K]
    W1 = np.asarray(W1, dtype=F32)
    W2 = np.asarray(W2, dtype=F32)
    bias = np.asarray(bias, dtype=F32)

    n_rows = x.shape[2]
    nsh = n_rows // n_cores
    nsh_pad = ((nsh + 15) // 16) * 16
    tiles = _tiles_for(nsh_pad)
    idx_all = np.asarray(ei[0, 0], dtype=np.int64)   # [N, K]
    assert idx_all.shape[1] == DEG_K

    x_cn = np.ascontiguousarray(x[0, :, :, 0])       # [C, N]
    x_bf16 = np.ascontiguousarray(x_cn.T).astype(BF16)
    x0_cn = np.ascontiguousarray(x_0[0].T)           # [C, N]

    deg = DEG_K + 1
    s1 = (1.0 - ALPHA) * (1.0 - BETA)
    s2 = ALPHA * (1.0 - BETA)
    eye = np.eye(C, dtype=np.float64)
    m1sT = ((s1 * eye + BETA * W1.astype(np.float64)).T / deg).astype(BF16)
    m2T = (s2 * eye + BETA * W2.astype(np.float64)).T.astype(BF16)
    bias_v = np.ascontiguousarray(bias.reshape(-1)[:, None].astype(F32))

    in_maps = []
    for c in range(n_cores):
        sl = slice(c * nsh, (c + 1) * nsh)
        table, idx16 = _prep_core(x_bf16, idx_all[sl], n_rows, nsh_pad, tiles)
        # xx columns interleaved per tile: [xself_t | x0_t] for each tile
        xx = np.zeros((P, 2 * nsh_pad), dtype=BF16)
        off = 0
        for nt in tiles:
            gsl = slice(c * nsh + off, c * nsh + min(off + nt, nsh))
            w = gsl.stop - gsl.start
            xx[:, 2 * off:2 * off + w] = x_cn[:, gsl]
            xx[:, 2 * off + nt:2 * off + nt + w] = x0_cn[:, gsl]
            off += nt
        in_maps.append(dict(
            tbl=table,
            idx=idx16,
            xx=xx,
            m1t=np.ascontiguousarray(m1sT),
            m2t=np.ascontiguousarray(m2T),
            biasv=bias_v,
        ))
    meta = dict(nsh=nsh, nsh_pad=nsh_pad, n_rows=n_rows, tiles=tiles)
    return in_maps, meta


last_results = None  # BassKernelResults of the most recent kernel() call
last_nc = None       # compiled Bass program of the most recent kernel() call


def kernel(x, x_0, edge_index, W1, W2, bias):
    global last_results, last_nc
    import os
    in_maps, meta = _prepare(x, x_0, edge_index, W1, W2, bias,
                             n_cores=N_CORES)
    nc = _build_program(meta["nsh_pad"], meta["tiles"])
    last_nc = nc
    trace = os.environ.get("GCNII_TRACE", "") == "1"
    try:
        res = run_bass_kernel_spmd(nc, in_maps, core_ids=list(range(N_CORES)),
                                   trace=trace)
    except ModuleNotFoundError:
        # BASS_TRACE=1 in an axon client without the NTFF profile hook dies
        # importing antenv.axon_hooks (before any execution); rerun untraced.
        os.environ["BASS_NEVER_TRACE"] = "1"
        res = run_bass_kernel_spmd(nc, in_maps, core_ids=list(range(N_CORES)),
                                   trace=False)
    last_results = res
    nsh = meta["nsh"]
    out = np.concatenate(
        [r["out"][:, :nsh].astype(F32) for r in res.results], axis=1)
    return np.ascontiguousarray(out)[None, :, :, None]


# --------------------------------------------------------------------------
# numpy model of the same math (for sim testing)
# --------------------------------------------------------------------------

def _numpy_reference(x, x_0, edge_index, W1, W2, bias):
    x2 = np.asarray(x, dtype=F32)[0, :, :, 0]            # [C, N]
    idx = np.asarray(edge_index)[0, 0]                   # [N, K]
    n = x2.shape[1]
    deg = idx.shape[1] + 1
    idx_full = np.concatenate([idx, np.arange(n)[:, None]], axis=1)
    x_j = x2[:, idx_full]                                # [C, N, K+1]
    aggr = x_j.sum(axis=-1) / deg                        # [C, N]
    aggr = aggr.T                                        # [N, C]
    x0 = np.asarray(x_0, dtype=F32)[0]
    s1 = (1.0 - ALPHA) * (1.0 - BETA)
    s2 = ALPHA * (1.0 - BETA)
    out = (aggr * s1 + aggr @ np.asarray(W1, dtype=F32).T * BETA
           + x0 * s2 + x0 @ np.asarray(W2, dtype=F32).T * BETA
           + np.asarray(bias, dtype=F32).reshape(1, -1))
    out = np.maximum(out, 0.0)
    return out.T[None, :, :, None]
